# revision 26
# baseline (speedup 1.0000x reference)
"""Trainium2 Bass kernel for nn_BaseMultiHeadAttention (B=2, S=2048, E=1024, H=16).

Sharding: tensor-parallel over heads - each of the 8 NeuronCores handles 2
heads for both batch elements (4 (b,h) jobs/core).  RMSNorm + RoPE + causal
attention run per-head on-device; the output projection is row-sharded
(each core contracts its 128 ctx features against proj_w), and the host
sums the 8 partial [B,S,E] outputs (the all-reduce) and adds the bias.

fp16 dataflow: the host ships q/k/v/cos/sin/proj_w as fp16 (halving input
DMA), phase-A elementwise math runs in fp16 on DVE (2x perf mode), the PE
transposes move fp16 data with an fp16 identity (1 cyc/row vs fp32's 2),
attention weights p = exp(s) are fp16, and the partial projection output is
shipped fp16 (host accumulates the 8 partials in f32).  Scores/ctx/proj
accumulate in f32 PSUM, RMSNorm sum-of-squares reduces in f32.

Per core pipeline:
  Phase A (per (b,h) job): DMA q/k [128, NT*64] fp16 (partition-major,
    rope-pair-deinterleaved by the host so every op is contiguous),
    sq=x*x (fp16 2x) -> reduce f32 -> sqrt on ACT -> recip on DVE ->
    xn = x*rs -> rope as A=xn*cosF, B=xn*sinF' (sign pre-baked), rn=A+swap(B)
    where swap is a negative-stride pair view -> PE transpose (fp16 ident)
    -> qT/kT [64, S] fp16 in SBUF.
  Phase B (per b, q-chunk of CW, head): scoresT[k,q] = kT.T @ qT (fp16,
    1cyc/row), block-causal; exp on ACT over [128,1024] PSUM groups (scale
    folded; no max subtraction: RMSNorm bounds |s*scale| <= 8); triangular
    mask on diag blocks; ctx = p.T @ [v|1] in PSUM (ones column -> row sums
    free), rows scaled by 1/sum on the PSUM->SBUF copy (fp16); PE-transpose
    ctx, partial projection vs fp16 weights, PSUM->SBUF fp16 copy, DMA out.
"""
import numpy as np

import bass_rust
import concourse.bass as bass
import concourse.mybir as mybir
import concourse.tile as tile
from concourse.bass_utils import run_bass_kernel_spmd
from concourse.masks import make_identity

B, S, E, H, D = 2, 2048, 1024, 16, 64
HD = D // 2
N_CORES = 8
HL = H // N_CORES          # 2 heads per core
NJ = B * HL                # 4 (b, h) jobs per core
NT = S // 128              # 16 s-tiles per job
EPS = 1.1920928955078125e-07
SCALE = float(D) ** -0.5
f32 = mybir.dt.float32
f32r = mybir.dt.float32r
fp16 = mybir.dt.float16
ALU = mybir.AluOpType
ACTF = mybir.ActivationFunctionType

# ---- tuning knobs -------------------------------------------------------
CW = 256                   # q-chunk width (256 trims exp waste vs 512)
NCH = S // CW              # chunks per job
QPC = CW // 128            # 128-row q-blocks per chunk
NBG = 1024 // CW           # k-blocks per [128,1024] exp group
MASK_ENGINE = "pool"       # "pool" affine_select | "dve" tri-mask multiply
OSB_ENGINES = ("dve", "dve", "act", "dve", "dve", "dve", "act", "dve")
QK_COPY_ENGINE = "dve"     # qT/kT PSUM->SBUF copies

_TC = tile.TileContext


def _legalize_waits(nc):
    """Split multi-wait sync_infos for this walrus build.

    This neuronxcc's codegen allows 1 sync wait per instruction (2 on
    EventSemaphore), while the Tile scheduler attaches all outstanding
    waits to one instruction.  Hoist the excess onto same-engine NoOps
    inserted immediately before the offending instruction - the engine
    executes its stream in order, so blocking semantics are identical.
    """
    uid = 0
    for f in nc.m.functions:
        for blk in f.blocks:
            insts = list(blk.instructions)
            out, changed = [], False
            for inst in insts:
                si = inst.sync_info
                cap = 2 if isinstance(inst, mybir.InstEventSemaphore) else 1
                if si is not None and len(si.on_wait) > cap:
                    changed = True
                    waits = list(si.on_wait)
                    for w in waits[:-cap]:
                        carrier = mybir.InstNoOp(
                            name=f"legwait-{uid}", engine=inst.engine,
                            ins=[], outs=[])
                        uid += 1
                        carrier.sync_info = bass_rust.SyncInfo(
                            on_wait=[w], on_update=[])
                        nc.register_instruction(carrier, overwrite=True)
                        out.append(carrier)
                    si.on_wait = waits[-cap:]
                    inst.sync_info = si
                out.append(inst)
            if changed:
                blk.instructions = out


def _swap_pairs(t, ns):
    """View of fp16 tile t [128, ns, 64] with the 32-wide half-pairs swapped
    along the last axis (negative-stride middle dim): element (p, s, 32*u+i)
    reads t[p, s, 32*(1-u)+i]."""
    return bass.AP(
        tensor=t.tensor, offset=t.offset + HD,
        ap=[list(t.ap[0]), [D, ns], [-HD, 2], [1, HD]])


def _bcast(t, ns, width):
    """Per-(partition,tile) scalar t [128, ns] broadcast along a new last
    axis of `width` (step-0 AP)."""
    return bass.AP(
        tensor=t.tensor, offset=t.offset,
        ap=[list(t.ap[0]), list(t.ap[1]), [0, width]])


def build_nc():
    nc = bass.Bass("TRN2", target_bir_lowering=False, debug=False)
    q_in = nc.dram_tensor("q", [NJ, 128, NT, D], fp16, kind="ExternalInput")
    k_in = nc.dram_tensor("k", [NJ, 128, NT, D], fp16, kind="ExternalInput")
    v_in = nc.dram_tensor("v", [NJ, 128, NT, D + 1], fp16,
                          kind="ExternalInput")
    cos_in = nc.dram_tensor("cos", [128, NT, D], fp16, kind="ExternalInput")
    sin_in = nc.dram_tensor("sin", [128, NT, D], fp16, kind="ExternalInput")
    wt_in = nc.dram_tensor("wt", [128, E], f32r, kind="ExternalInput")
    out = nc.dram_tensor("out", [B * S, E], fp16, kind="ExternalOutput")

    with _TC(nc) as tc:
        with tc.tile_pool(name="const", bufs=1) as cp, \
             tc.tile_pool(name="pa", bufs=3) as pa, \
             tc.tile_pool(name="pb", bufs=2) as pb, \
             tc.tile_pool(name="pc", bufs=18) as pc, \
             tc.tile_pool(name="pr", bufs=1) as pr, \
             tc.tile_pool(name="pp", bufs=12) as pp, \
             tc.tile_pool(name="po", bufs=6) as po, \
             tc.tile_pool(name="ps_s", bufs=2, space="PSUM") as ps_s, \
             tc.tile_pool(name="ps_sm", bufs=2, space="PSUM") as ps_sm, \
             tc.tile_pool(name="ps_o", bufs=2, space="PSUM") as ps_o:
            ident32 = cp.tile([128, 128], f32)
            make_identity(nc, ident32)
            ident = ident32.bitcast(f32r)
            # tri_neg[r, c] = -1e30 for r < c else 0; adding tri_neg to the
            # diagonal score block (via lhsT=ident matmul) kills q < k entries
            # before the exp, so no post-exp masking pass is needed.
            identf = cp.tile([128, 128], fp16)
            make_identity(nc, identf)
            tri_neg = cp.tile([128, 128], fp16)
            nc.vector.memset(tri_neg, -60000.0)
            nc.gpsimd.affine_select(
                out=tri_neg, in_=tri_neg, compare_op=ALU.is_gt, fill=0.0,
                base=0, pattern=[[1, 128]], channel_multiplier=-1)
            eps_t = cp.tile([128, 1], f32)
            nc.vector.memset(eps_t, EPS)
            cos_sb = cp.tile([128, NT, D], fp16)
            sin_sb = cp.tile([128, NT, D], fp16)
            wt_sb = cp.tile([128, E], f32r)
            qT = cp.tile([64, NJ, S], f32r)
            kT = cp.tile([64, NJ, S], f32r)
            vsb = cp.tile([128, NJ, NT, D + 1], fp16)
            cpair = {}                        # (b, c) -> [QPC tiles]

            # ---- Phase A piece: norm + rope + transpose of one span --------
            raws = {}

            def load_raw(j, which, sub, nsub=2):
                NS = NT // nsub
                src = q_in if which == "q" else k_in
                tsl = slice(sub * NS, (sub + 1) * NS)
                raw = pr.tile([128, NS, D], fp16, tag=f"raw{j}{which}{sub}",
                              name="raw")
                nc.sync.dma_start(out=raw, in_=src.ap()[j][:, tsl])
                raws[(j, which, sub)] = raw

            def phase_a(j, which, sub, nsub=2):
                NS = NT // nsub
                dstT = qT if which == "q" else kT
                tsl = slice(sub * NS, (sub + 1) * NS)
                raw = raws.pop((j, which, sub))
                sq = pa.tile([128, NS, D], fp16, tag="sq", name="sq")
                nc.vector.tensor_mul(sq, raw, raw)
                ss8 = pa.tile([128, NS, 8], fp16, tag="ss8", name="ss8")
                with nc.allow_low_precision(
                        reason="inner sum of 8 squares; fp16 ok"):
                    nc.vector.reduce_sum(
                        ss8, sq.rearrange("p t (a b) -> p t a b", b=8),
                        axis=mybir.AxisListType.X)
                ss = pa.tile([128, NS], f32, tag="ss", name="ss")
                nc.vector.reduce_sum(ss, ss8, axis=mybir.AxisListType.X)
                rs = pa.tile([128, NS], fp16, tag="rs", name="rs")
                nc.scalar.activation(
                    out=rs, in_=ss, func=ACTF.Sqrt, bias=eps_t, scale=1.0 / D)
                with nc.allow_low_precision(
                        reason="rsqrt of mean-square ~1; fp16 ok"):
                    nc.vector.reciprocal(out=rs, in_=rs)
                xn = pa.tile([128, NS, D], fp16, tag="xn", name="xn")
                nc.vector.tensor_mul(xn, raw, _bcast(rs, NS, D))
                ra = pa.tile([128, NS, D], fp16, tag="ra", name="ra")
                rb = pa.tile([128, NS, D], fp16, tag="rb", name="rb")
                nc.vector.tensor_mul(ra, xn, cos_sb[:, tsl, :])
                nc.vector.tensor_mul(rb, xn, sin_sb[:, tsl, :])
                rn = pa.tile([128, NS, D], f32r, tag="rn", name="rn")
                nc.vector.tensor_add(rn, ra, _swap_pairs(rb, NS))
                for tg in range(NS // 4):
                    ps_tr = ps_o.tile([64, 512], f32r, tag="o", name="ps_tr")
                    for tt in range(4):
                        t = tg * 4 + tt
                        nc.tensor.transpose(
                            ps_tr[:, tt * 128:(tt + 1) * 128],
                            rn[:, t, :], ident)
                    base = (sub * NS + tg * 4) * 128
                    nc.vector.tensor_copy(dstT[:, j, base:base + 512], ps_tr)

            def load_v(j):
                # host ships v with the ones column baked in
                nc.sync.dma_start(out=vsb[:, j], in_=v_in.ap()[j])

            # ---- attention for one (batch, local head, chunk) --------------
            def attn(b, hl, c):
                j = b * HL + hl
                nkb = (c + 1) * QPC      # causal k-blocks for this chunk
                ngrp = (nkb + NBG - 1) // NBG
                if (b, c) not in cpair:
                    cpair[(b, c)] = [
                        pc.tile([128, HL * D], f32r, tag=f"cpair{qb}",
                                name=f"cpair{qb}")
                        for qb in range(QPC)]
                cp_row = cpair[(b, c)]
                ptiles = []
                for g in range(ngrp):
                    nb = min(NBG, nkb - g * NBG)
                    sps = ps_s.tile([128, 1024], f32, tag="s", name="sps")
                    for u in range(nb):
                        jj = g * NBG + u
                        jl = jj - c * QPC
                        nc.tensor.matmul(
                            sps[:, u * CW:(u + 1) * CW],
                            lhsT=kT[:, j, jj * 128:(jj + 1) * 128],
                            rhs=qT[:, j, c * CW:(c + 1) * CW],
                            start=True, stop=(jl < 0),
                        )
                        if jl >= 0:
                            # diag block: add -inf upper triangle pre-exp
                            nc.tensor.matmul(
                                sps[:, u * CW + jl * 128:
                                    u * CW + (jl + 1) * 128],
                                lhsT=identf, rhs=tri_neg,
                                start=False, stop=True,
                                skip_group_check=True,
                            )
                    pt = pp.tile([128, 1024], fp16, tag="p", name="pt")
                    nc.scalar.activation(
                        out=pt[:, 0:nb * CW], in_=sps[:, 0:nb * CW],
                        func=ACTF.Exp, scale=SCALE)
                    ptiles.append(pt)
                ctx = ps_sm.tile([128, QPC, D + 1], f32, tag="sm", name="ctx")
                for qb in range(QPC):
                    i = c * QPC + qb
                    for jj in range(i + 1):
                        g, u = jj // NBG, jj % NBG
                        nc.tensor.matmul(
                            ctx[:, qb, :],
                            lhsT=ptiles[g][:, u * CW + qb * 128:
                                           u * CW + (qb + 1) * 128],
                            rhs=vsb[:, j, jj, :],
                            start=(jj == 0), stop=(jj == i),
                        )
                    rsum = pb.tile([128, 1], f32, tag="rsum", name="rsum")
                    nc.vector.reciprocal(out=rsum, in_=ctx[:, qb, D:D + 1])
                    nc.vector.tensor_scalar_mul(
                        cp_row[qb][:, hl * D:(hl + 1) * D],
                        ctx[:, qb, 0:D], rsum)

            # ---- projection for one (batch, chunk): needs both heads -------
            def proj(b, c, oengs=None):
                cp_row = cpair.pop((b, c))
                trp = ps_sm.tile([128, QPC * 128], f32r, tag="sm", name="trp")
                for qb in range(QPC):
                    nc.tensor.transpose(
                        trp[:, qb * 128:(qb + 1) * 128], cp_row[qb], ident)
                ctxT2 = pb.tile([128, QPC * 128], f32r, tag="ctxT",
                                name="ctxT")
                nc.vector.tensor_copy(ctxT2, trp)
                for qb in range(QPC):
                    i = c * QPC + qb
                    ctxT = ctxT2[:, qb * 128:(qb + 1) * 128]
                    osb = po.tile([128, E], fp16, tag="osb", name="osb")
                    for n in range(2):
                        ops_ = ps_o.tile([128, 512], f32, tag="o", name="ops")
                        nc.tensor.matmul(
                            ops_, lhsT=ctxT,
                            rhs=wt_sb[:, n * 512:(n + 1) * 512],
                            start=True, stop=True,
                        )
                        cyc = oengs or OSB_ENGINES
                        oeng = cyc[(2 * i + n) % len(cyc)]
                        if oeng == "act":
                            nc.scalar.copy(
                                out=osb[:, n * 512:(n + 1) * 512], in_=ops_)
                        else:
                            eng = {"pool": nc.gpsimd, "dve": nc.vector}[oeng]
                            eng.tensor_copy(
                                osb[:, n * 512:(n + 1) * 512], ops_)
                    nc.sync.dma_start(
                        out=out.ap()[b * S + i * 128:b * S + (i + 1) * 128, :],
                        in_=osb,
                    )

            nc.sync.dma_start(out=cos_sb, in_=cos_in.ap())
            nc.sync.dma_start(out=sin_sb, in_=sin_in.ap())
            nc.sync.dma_start(out=wt_sb, in_=wt_in.ap())
            # all input DMAs issued upfront: no producers, so the SP queue
            # streams them immediately; phase-A pieces just read SBUF.
            for j in range(NJ):
                for which in ("q", "k"):
                    for s_ in range(2):
                        load_raw(j, which, s_)
                load_v(j)
            # head-0 attention starts right after job 0's phase A; later
            # phase-A pieces slot between attention calls so DVE phase-A
            # work hides under ACT/PE attention work.
            phase_a(0, "q", 0); phase_a(0, "k", 0)
            phase_a(0, "q", 1); phase_a(0, "k", 1)
            attn(0, 0, 0); attn(0, 0, 1); attn(0, 0, 2)
            phase_a(1, "q", 0); phase_a(1, "q", 1)
            attn(0, 0, 3); attn(0, 0, 4)
            phase_a(1, "k", 0); phase_a(1, "k", 1)
            attn(0, 0, 5); attn(0, 0, 6); attn(0, 0, 7)
            attn(0, 1, 0); proj(0, 0)
            attn(0, 1, 1); proj(0, 1)
            attn(0, 1, 2); proj(0, 2)
            phase_a(2, "q", 0); phase_a(2, "q", 1)
            attn(0, 1, 3); proj(0, 3)
            attn(0, 1, 4); proj(0, 4)
            phase_a(2, "k", 0); phase_a(2, "k", 1)
            attn(0, 1, 5); proj(0, 5)
            attn(0, 1, 6); proj(0, 6)
            attn(1, 0, 7)
            attn(0, 1, 7); proj(0, 7)
            phase_a(3, "q", 0); phase_a(3, "q", 1)
            attn(1, 0, 6)
            phase_a(3, "k", 0); phase_a(3, "k", 1)
            attn(1, 0, 5)
            attn(1, 1, 7); proj(1, 7)
            attn(1, 0, 4); attn(1, 1, 6); proj(1, 6)
            attn(1, 0, 3); attn(1, 1, 5); proj(1, 5)
            attn(1, 0, 2); attn(1, 1, 4); proj(1, 4)
            attn(1, 0, 1); attn(1, 1, 3); proj(1, 3)
            attn(1, 0, 0); attn(1, 1, 2); proj(1, 2)
            attn(1, 1, 1); proj(1, 1, ("act", "dve", "dve", "act"))
            attn(1, 1, 0); proj(1, 0, ("act", "dve", "dve", "act"))
    _legalize_waits(nc)
    return nc


# even rope lanes first, then odd - see Phase A comment
_ROPE_PERM = np.concatenate([np.arange(0, D, 2), np.arange(1, D, 2)])


def _shard_inputs(q, k, v, cos, sin, proj_w):
    """Per-core input maps (host-side layout prep only - no module math)."""
    qh = q.reshape(B, S, H, D)
    kh = k.reshape(B, S, H, D)
    vh = v.reshape(B, S, H, D)
    # rope tables: full width [S, D] with cos duplicated and the sign of the
    # second sin half flipped (so rope is A + swap(B)); partition-major.
    cosF = np.concatenate([cos, cos], axis=1).astype(np.float16)
    sinF = np.concatenate([sin, -sin], axis=1).astype(np.float16)
    cos_t = np.ascontiguousarray(
        cosF.reshape(NT, 128, D).transpose(1, 0, 2))
    sin_t = np.ascontiguousarray(
        sinF.reshape(NT, 128, D).transpose(1, 0, 2))
    maps = []
    for core in range(N_CORES):
        hs = slice(HL * core, HL * (core + 1))

        def tiles(x, permute, pad_ones=False):
            xs = x[:, :, hs, :].transpose(0, 2, 1, 3)  # [B, HL, S, D]
            if permute:
                xs = xs[..., _ROPE_PERM]
            if pad_ones:
                xs = np.concatenate(
                    [xs, np.ones(xs.shape[:-1] + (1,), xs.dtype)], axis=-1)
            d = xs.shape[-1]
            # [NJ, NT, 128, d] -> [NJ, 128, NT, d] partition-major
            return np.ascontiguousarray(
                xs.reshape(NJ, NT, 128, d).transpose(0, 2, 1, 3)
            ).astype(np.float16)

        wt_c = np.ascontiguousarray(
            proj_w[:, 128 * core:128 * (core + 1)].T).astype(np.float32)
        maps.append({
            "q": tiles(qh, True), "k": tiles(kh, True),
            "v": tiles(vh, False, pad_ones=True),
            "cos": cos_t, "sin": sin_t, "wt": wt_c,
        })
    return maps


_NC_CACHE = []


def _get_nc():
    if not _NC_CACHE:
        _NC_CACHE.append(build_nc())
    return _NC_CACHE[0]


def kernel(q, k, v, attn_mask, padding_mask, qn_w, kn_w, proj_w, proj_b,
           cos, sin):
    q = np.asarray(q, np.float32)
    k = np.asarray(k, np.float32)
    v = np.asarray(v, np.float32)
    proj_w = np.asarray(proj_w, np.float32)
    proj_b = np.asarray(proj_b, np.float32)
    cos = np.asarray(cos, np.float32)
    sin = np.asarray(sin, np.float32)
    attn_mask = np.asarray(attn_mask)
    padding_mask = np.asarray(padding_mask)
    qn_w = np.asarray(qn_w, np.float32)
    kn_w = np.asarray(kn_w, np.float32)
    # The kernel bakes in: causal attn_mask, no padding, unit RMSNorm weights.
    assert np.array_equal(
        attn_mask.reshape(S, S), np.tril(np.ones((S, S), attn_mask.dtype)))
    assert padding_mask.all()
    assert np.all(qn_w == 1.0) and np.all(kn_w == 1.0)

    in_maps = _shard_inputs(q, k, v, cos, sin, proj_w)
    nc = _get_nc()
    res = run_bass_kernel_spmd(nc, in_maps, core_ids=list(range(N_CORES)))
    parts = np.stack([r["out"] for r in res.results])      # [8, B*S, E] fp16
    full = parts.astype(np.float32).sum(axis=0) + proj_b[None, :]
    return full.reshape(B, S, E).astype(np.float32)
